# revision 53
# baseline (speedup 1.0000x reference)
"""CrossFocusedLinearAttentionPrune kernel for 8x TRN2 NeuronCores.

Data-parallel over batch B=8: one batch element per core; the small CxC
weights / C-vectors are replicated (host pre-transposed + pre-cast).

Per-core pipeline (v2):
  - host uploads qT/kT channel-major bf16 and v row-major bf16 (no on-device
    input transposes); 1/softplus(scale) folded into Wq/Wk (relu(x)/s ==
    relu(x/s) for s>0; the 1e-6 eps is dropped, far below bf16 resolution)
  - K path row-major: per 128-row tile, proj -> relu -> cube; kv uses the
    reassociation kv = (k3^T v_raw) Wv^T so v needs no projection pass;
    ksum rides along as a ones-column matmul into the same PSUM group
  - Q path channel-major: proj -> relu -> cube -> q3 [c, n]
  - z: ksum replicated along free dim -> one matmul per chunk gives z_num
    broadcast across all 128 partitions; Act Reciprocal(x+eps) -> zrep
  - x channel-major directly (lhsT = kv), eviction fuses the z multiply and
    writes fp8 into the zero-padded 68x68 conv map
  - depthwise 5x5 conv: 13 fp8 DoubleRow matmuls per c-block (tap pairs via
    a custom strided AP; 26th tap = dwc bias against an all-ones region)
  - h = conv + q3 (gpsimd); out = h @ Wproj^T + bproj, bias via a rank-1
    ones matmul, output DMA'd straight from PSUM per 128-row tile
"""

import os

import numpy as np
import ml_dtypes

import concourse.bacc as bacc
import concourse.bass as bass
import concourse.mybir as mybir
import concourse.tile as tile
from concourse.ap import AP
from concourse.bass_utils import run_bass_kernel_spmd

F32 = mybir.dt.float32
BF16 = mybir.dt.bfloat16
FP8 = mybir.dt.float8e4
AF = mybir.ActivationFunctionType
ALU = mybir.AluOpType
PERF2 = mybir.MatmulPerfMode.DoubleRow

B, N, C = 8, 4096, 256
H = W = 64
KS, PAD = 5, 2
HP = H + 2 * PAD          # 68
EPS = 1e-6
CT = 2                    # channel tiles of 128
NCH = 8                   # 512-wide chunks over N
CHUNK = 512
NT = 32                   # 128-row tiles over N
GUARD = 4                 # zero guard before each ct's map region
ONES_OFF = HP * HP        # ones region for the bias tap (within map part)
XFREE = HP * HP + 544     # per-ct free size incl ones region (excl guard)
XTOT = GUARD + XFREE
NPAIR = 13                # 25 taps + bias tap = 13 DoubleRow pairs
BF16NP = ml_dtypes.bfloat16
FP8NP = ml_dtypes.float8_e4m3

# taps t = 0..24 -> (dy, dx) = (t//5 - 2, t%5 - 2); t = 25 -> bias tap.
# DoubleRow windows must be 2-byte aligned in the fp8 map, so taps are
# paired by dx parity: even-dx taps (and the bias tap) stream full-width
# contiguous [272] windows; odd-dx taps go per map-row with a +1 psum
# shift so their bases become even.
TAPS_EVEN = [t for t in range(25) if (t % 5) % 2 == 0] + [25]   # 16
TAPS_ODD = [t for t in range(25) if (t % 5) % 2 == 1]           # 10
PAIRS_EVEN = [(TAPS_EVEN[2 * i], TAPS_EVEN[2 * i + 1]) for i in range(8)]
PAIRS_ODD = [(TAPS_ODD[2 * i], TAPS_ODD[2 * i + 1]) for i in range(5)]


def _tap_base(t, ch, half):
    # window base so that psum col 68*r + 2 + c maps to out pixel
    # (8*ch + 4*half + r, c); base = (i0 + 2 + dy)*68 + dx
    dy, dx = t // 5 - 2, t % 5 - 2
    return (8 * ch + 4 * half + 2 + dy) * HP + dx


def build_program():
    nc = bacc.Bacc("TRN2", target_bir_lowering=False, debug=False,
                   enable_asserts=False, num_devices=8)

    # -------- DRAM tensors (per-core inputs) --------
    qT_d = nc.dram_tensor("qT", [C, N], BF16, kind="ExternalInput").ap()
    kT_d = nc.dram_tensor("kT8", [C, N], FP8, kind="ExternalInput").ap()
    v_d = nc.dram_tensor("v8", [128, NT * C], FP8, kind="ExternalInput").ap()
    wq_d = nc.dram_tensor("wqTs", [C, C], BF16, kind="ExternalInput").ap()
    wk_d = nc.dram_tensor("wkTs", [C, C], BF16, kind="ExternalInput").ap()
    wv_d = nc.dram_tensor("wvT", [C, C], BF16, kind="ExternalInput").ap()
    wp_d = nc.dram_tensor("wpT", [C, C], BF16, kind="ExternalInput").ap()
    d8_d = nc.dram_tensor("diag8", [128, CT * NPAIR * 2 * 128], FP8,
                          kind="ExternalInput").ap()
    id_d = nc.dram_tensor("ident", [128, 128], BF16, kind="ExternalInput").ap()
    bp_d = nc.dram_tensor("bprow", [1, C], BF16, kind="ExternalInput").ap()
    out_d = nc.dram_tensor("out", [N, C], BF16, kind="ExternalOutput").ap()

    v_r = v_d.rearrange("p (nt c) -> p nt c", c=C)
    out_r = out_d.rearrange("(nt p) c -> p nt c", p=128)

    with tile.TileContext(nc) as tc:
        with (
            tc.tile_pool(name="const", bufs=1) as const,
            tc.tile_pool(name="big", bufs=1) as big,
            tc.tile_pool(name="kpool", bufs=6) as kpool,
            tc.tile_pool(name="qpool", bufs=6) as qpool,
            tc.tile_pool(name="zpool", bufs=4) as zpool,
            tc.tile_pool(name="hpool", bufs=4) as hpool,
            tc.tile_pool(name="smal", bufs=1) as smal,
            tc.tile_pool(name="psA", bufs=2, space="PSUM") as psA,
            tc.tile_pool(name="psB", bufs=2, space="PSUM") as psB,
            tc.tile_pool(name="psC", bufs=2, space="PSUM") as psC,
            tc.tile_pool(name="psKV", bufs=1, space="PSUM") as psKV,
            tc.tile_pool(name="psS", bufs=1, space="PSUM") as psS,
        ):
            # -------- K/V-critical constants, then inputs, then the rest ----
            wk_sb = const.tile([128, CT, C], BF16)
            nc.sync.dma_start(wk_sb[:], wk_d.rearrange("(ct p) d -> p ct d", p=128))
            wq_sb = const.tile([128, CT, C], BF16)
            nc.sync.dma_start(wq_sb[:], wq_d.rearrange("(ct p) d -> p ct d", p=128))

            kT_sb = big.tile([128, CT, N], FP8)
            qT_sb = big.tile([128, CT, N], BF16)
            v_sb = big.tile([128, NT, C], FP8)
            kT_r = kT_d.rearrange("(ct p) n -> p ct n", p=128)
            qT_r = qT_d.rearrange("(ct p) n -> p ct n", p=128)
            kt_cuts = [0, 256, 512] + [512 * i for i in range(2, 9)]
            for i in range(len(kt_cuts) - 1):
                s = slice(kt_cuts[i], kt_cuts[i + 1])
                nc.sync.dma_start(kT_sb[:, :, s], kT_r[:, :, s])
                j = i - 1
                if 0 <= j < 8:
                    nc.sync.dma_start(v_sb[:, 4 * j:4 * j + 4, :],
                                      v_r[:, 4 * j:4 * j + 4, :])
                if 4 <= j < 8:
                    s2 = slice((j - 4) * N // 8, (j - 3) * N // 8)
                    nc.sync.dma_start(qT_sb[:, :, s2], qT_r[:, :, s2])
            for i in range(4, 8):
                s = slice(i * N // 8, (i + 1) * N // 8)
                nc.sync.dma_start(qT_sb[:, :, s], qT_r[:, :, s])

            wv_sb = const.tile([128, CT, C], BF16)
            nc.sync.dma_start(wv_sb[:], wv_d.rearrange("(ct p) d -> p ct d", p=128))
            wp_sb = const.tile([128, CT, C], BF16)
            nc.sync.dma_start(wp_sb[:], wp_d.rearrange("(ct p) d -> p ct d", p=128))
            d8_sb = const.tile([128, CT * NPAIR * 2 * 128], FP8)
            nc.sync.dma_start(d8_sb[:], d8_d)
            id_sb = const.tile([128, 128], BF16)
            nc.sync.dma_start(id_sb[:], id_d)
            bp_sb = const.tile([1, C], BF16)
            nc.sync.dma_start(bp_sb[:], bp_d)

            ones1 = smal.tile([1, 128], BF16)
            nc.vector.memset(ones1[:], 1.0)
            onesk = smal.tile([128, 2, 1], FP8)
            nc.vector.memset(onesk[:], 1.0)
            ones128 = smal.tile([128, 128], BF16)
            nc.vector.memset(ones128[:], 1.0)
            epsrow = smal.tile([1, CHUNK], BF16)
            nc.vector.memset(epsrow[:], EPS)

            # -------- persistent tensors --------
            q3 = big.tile([128, CT, N], BF16)
            xpad = big.tile([128, CT, XTOT], FP8)
            kv_sb = smal.tile([128, CT, C], BF16)
            ksum_bf = smal.tile([128, CT], F32)
            ksr = smal.tile([128, CT, 128], BF16)

            # conv-map halo zeros + ones region (borders only; interior is
            # fully overwritten by the x-phase evictions)
            xmaps = [xpad[:, dt, GUARD:GUARD + HP * HP]
                     .rearrange("p (r c) -> p r c", c=HP) for dt in range(CT)]
            for dt in range(CT):
                xm = xmaps[dt]
                nc.vector.memset(xpad[:, dt, 0:GUARD], 0.0)         # guard
                nc.vector.memset(xm[:, 0:2, :], 0.0)                # top rows
                nc.vector.memset(xm[:, 2 + H:2 + H + 2, :], 0.0)    # bottom
                nc.vector.memset(xm[:, 2:2 + H, 0:2], 0.0)          # left
                nc.vector.memset(xm[:, 2:2 + H, 2 + W:HP], 0.0)     # right
                nc.vector.memset(xpad[:, dt, GUARD + ONES_OFF:XTOT], 1.0)

            kv_ps = psKV.tile([128, CT, C], F32, name="kvps")
            ks_ps = psS.tile([128, CT, 1], F32, name="ksps")

            # ============ K/V phase (row-major) + interleaved Q ============
            def q_chunk(ch):
                # back-half chunks run during the PE-bound conv phase, where
                # DVE is loaded but gpsimd is idle
                eng = nc.vector if ch < 4 else nc.gpsimd
                for dt in range(CT):
                    qps = psA.tile([128, CHUNK], F32, tag="a")
                    for ct in range(CT):
                        nc.tensor.matmul(qps[:], lhsT=wq_sb[:, ct, dt * 128:(dt + 1) * 128],
                                         rhs=qT_sb[:, ct, ch * CHUNK:(ch + 1) * CHUNK],
                                         start=(ct == 0), stop=(ct == 1))
                    mq = qpool.tile([128, CHUNK], BF16, tag="mq")
                    nc.scalar.activation(mq[:], qps[:], AF.Relu)
                    mq2 = qpool.tile([128, CHUNK], BF16, tag="mq2")
                    eng.tensor_tensor(mq2[:], mq[:], mq[:], op=ALU.mult)
                    eng.tensor_tensor(q3[:, dt, ch * CHUNK:(ch + 1) * CHUNK],
                                      mq2[:], mq[:], op=ALU.mult)

            k3_map = {}

            def kv_pair(mm):
                k3 = k3_map.pop(mm)
                for dt in range(CT):
                    blk = k3[:, :, dt * 128:(dt + 1) * 128]
                    nc.tensor.matmul(kv_ps[:, dt, :], lhsT=blk,
                                     rhs=v_sb[:, 2 * mm:2 * mm + 2, :],
                                     start=(mm == 0), stop=(mm == NT // 2 - 1),
                                     perf_mode=PERF2)
                    nc.tensor.matmul(ks_ps[:, dt, :], lhsT=blk, rhs=onesk[:],
                                     start=(mm == 0), stop=(mm == NT // 2 - 1),
                                     perf_mode=PERF2)

            # K processed two 128-row tiles at a time (one [128, 512] psum) to
            # halve the relu/cube op count
            for mm in range(NT // 2):
                kps = psB.tile([128, 2, C], F32, tag="b")
                for g in range(2):
                    m = 2 * mm + g
                    for ct in range(CT):
                        nc.tensor.matmul(kps[:, g, :],
                                         lhsT=kT_sb[:, ct, m * 128:(m + 1) * 128],
                                         rhs=wk_sb[:, ct, :], start=(ct == 0), stop=(ct == 1))
                mk = kpool.tile([128, 2 * C], BF16, tag="mk")
                nc.scalar.activation(mk[:], kps.rearrange("p g c -> p (g c)"), AF.Relu)
                mk2 = kpool.tile([128, 2 * C], BF16, tag="mk2")
                nc.vector.tensor_tensor(mk2[:], mk[:], mk[:], op=ALU.mult)
                k3 = kpool.tile([128, 2, C], FP8, tag="k3", name=f"k3_{mm}")
                k3eng = nc.gpsimd if mm % 2 == 0 else nc.vector
                k3eng.tensor_tensor(k3.rearrange("p g c -> p (g c)"),
                                    mk2[:], mk[:], op=ALU.mult)
                k3_map[mm] = k3
                # kv/ksum contraction lags one mm-step so the relu/cube chain
                # never gates PE; two m-tiles per DoubleRow matmul
                if mm >= 1:
                    kv_pair(mm - 1)
                if mm % 2 == 1 and mm // 2 < 3:
                    q_chunk(mm // 2)
            kv_pair(NT // 2 - 1)

            # ============ ksum replicate + kv fix-up ============
            # q_chunk(3) lands here so PE has work under the Act/DVE links
            nc.scalar.copy(ksum_bf[:], ks_ps.rearrange("p ct one -> p (ct one)"))
            q_chunk(3)
            for dt in range(CT):
                nc.vector.tensor_scalar(ksr[:, dt, :], ones128[:],
                                        ksum_bf[:, dt:dt + 1], None, op0=ALU.mult)

            zrep_map = {}

            def z_chunk(ch):
                zps = psB.tile([128, CHUNK], F32, tag="b")
                nc.tensor.matmul(zps[:], lhsT=ones1[:], rhs=epsrow[:],
                                 start=True, stop=False)
                for ct in range(CT):
                    nc.tensor.matmul(zps[:], lhsT=ksr[:, ct, :],
                                     rhs=q3[:, ct, ch * CHUNK:(ch + 1) * CHUNK],
                                     start=False, stop=(ct == 1))
                zrep = zpool.tile([128, CHUNK], BF16, tag="z", name=f"z{ch}")
                with nc.allow_low_precision(reason="z broadcast, conv-term only"):
                    nc.vector.reciprocal(zrep[:], zps[:])
                zrep_map[ch] = zrep

            # z for the first chunks front-runs the kv fix-up chain so PE
            # stays busy during its Act/DVE links
            z_chunk(0)
            z_chunk(1)

            tmp_sb = smal.tile([128, CT, C], BF16)
            for dt in range(CT):
                nc.scalar.copy(tmp_sb[:, dt, :], kv_ps[:, dt, :])
            tmpT = smal.tile([128, CT, C], BF16)   # [e, ct(c-blk), c]
            for eb in range(CT):
                for cb in range(CT):
                    tps = psB.tile([128, 128], BF16, tag="b")
                    nc.tensor.transpose(tps[:], tmp_sb[:, cb, eb * 128:(eb + 1) * 128],
                                        id_sb[:])
                    nc.vector.tensor_copy(tmpT[:, eb, cb * 128:(cb + 1) * 128], tps[:])
            for cb in range(CT):
                kvps = psA.tile([128, C], F32, tag="a")
                for eb in range(CT):
                    nc.tensor.matmul(kvps[:], lhsT=tmpT[:, eb, cb * 128:(cb + 1) * 128],
                                     rhs=wv_sb[:, eb, :], start=(eb == 0), stop=(eb == 1))
                nc.scalar.copy(kv_sb[:, cb, :], kvps[:])

            # ============ z + x + conv + proj pipeline ============
            xpad_h = xpad[:, 0, 0:1]   # handle for custom-stride APs
            PSTRIDE = CT * XTOT

            def _pair_rhs(o0, o1, width):
                return AP(xpad_h.tensor, o0,
                          [[PSTRIDE, 128], [o1 - o0, 2], [1, width]])

            hch_map = {}

            def conv_half(ch, half):
                if half == 0:
                    hch_map[ch] = hpool.tile([128, CT, CHUNK], BF16, tag="h",
                                             name=f"h{ch}")
                hch = hch_map[ch]
                for dt in range(CT):
                    if True:
                        base = dt * XTOT + GUARD
                        cps = psC.tile([128, 273], F32, tag="cv")

                        def off(t):
                            if t == 25:
                                return base + ONES_OFF
                            return base + _tap_base(t, ch, half)

                        nmm = 8 + 5 * 4
                        i = 0
                        for j, (ta, tb) in enumerate(PAIRS_EVEN):
                            lhsT = d8_sb[:, (dt * NPAIR + j) * 256:
                                         (dt * NPAIR + j + 1) * 256]
                            nc.tensor.matmul(
                                cps[:, 0:272], lhsT=lhsT.rearrange("p (two m) -> p two m", two=2),
                                rhs=_pair_rhs(off(ta), off(tb), 272),
                                start=(i == 0), stop=(i == nmm - 1),
                                perf_mode=PERF2, skip_group_check=True)
                            i += 1
                        for j, (ta, tb) in enumerate(PAIRS_ODD):
                            lhsT = d8_sb[:, (dt * NPAIR + 8 + j) * 256:
                                         (dt * NPAIR + 8 + j + 1) * 256]
                            lv = lhsT.rearrange("p (two m) -> p two m", two=2)
                            for r in range(4):
                                sh = HP * r + 1
                                nc.tensor.matmul(
                                    cps[:, sh:sh + HP], lhsT=lv,
                                    rhs=_pair_rhs(off(ta) + sh, off(tb) + sh, HP),
                                    start=False, stop=(i == nmm - 1),
                                    perf_mode=PERF2, skip_group_check=True)
                                i += 1
                        cv = cps[:, 0:272].rearrange("p (r c) -> p r c", c=HP)
                        hv = hch[:, dt, half * 256:(half + 1) * 256] \
                            .rearrange("p (r c) -> p r c", c=W)
                        qv = q3[:, dt, ch * CHUNK + half * 256:
                                ch * CHUNK + (half + 1) * 256] \
                            .rearrange("p (r c) -> p r c", c=W)
                        nc.vector.tensor_tensor(hv, cv[:, :, 2:2 + W], qv, op=ALU.add)

            def proj_half(ch, half):
                hch = hch_map[ch] if half == 0 else hch_map.pop(ch)
                ostage = hpool.tile([128, 2, C], BF16, tag="os")
                last = (ch == NCH - 1)
                for g in (2 * half, 2 * half + 1):
                    ops = psB.tile([128, C], F32, tag="b")
                    nc.tensor.matmul(ops[:], lhsT=ones1[:], rhs=bp_sb[:],
                                     start=True, stop=False)
                    for ct in range(CT):
                        nc.tensor.matmul(ops[:], lhsT=hch[:, ct, g * 128:(g + 1) * 128],
                                         rhs=wp_sb[:, ct, :], start=False, stop=(ct == 1))
                    nc.scalar.copy(ostage[:, g - 2 * half, :], ops[:])
                    if last:
                        nc.sync.dma_start(out_r[:, 4 * ch + g, :],
                                          ostage[:, g - 2 * half, :])
                if not last:
                    nc.sync.dma_start(out_r[:, 4 * ch + 2 * half:4 * ch + 2 * half + 2, :],
                                      ostage[:])

            for ch in range(NCH):
                if ch + 4 < NCH:
                    q_chunk(ch + 4)   # q3 for the back half, off the A/B phase
                zrep = zrep_map.pop(ch)
                zv = zrep.rearrange("p (r c) -> p r c", c=W)
                for dt in range(CT):
                    xps = psA.tile([128, CHUNK], F32, tag="a")
                    for ct in range(CT):
                        nc.tensor.matmul(xps[:], lhsT=kv_sb[:, ct, dt * 128:(dt + 1) * 128],
                                         rhs=q3[:, ct, ch * CHUNK:(ch + 1) * CHUNK],
                                         start=(ct == 0), stop=(ct == 1))
                    nc.vector.tensor_tensor(
                        xmaps[dt][:, 2 + 8 * ch:2 + 8 * ch + 8, 2:2 + W],
                        xps.rearrange("p (r c) -> p r c", c=W), zv, op=ALU.mult)
                if ch + 2 < NCH:
                    z_chunk(ch + 2)
                if ch >= 2:
                    conv_half(ch - 2, 1)
                    proj_half(ch - 2, 1)
                if ch >= 1:
                    conv_half(ch - 1, 0)
                    proj_half(ch - 1, 0)
            conv_half(NCH - 2, 1)
            proj_half(NCH - 2, 1)
            conv_half(NCH - 1, 0)
            proj_half(NCH - 1, 0)
            conv_half(NCH - 1, 1)
            proj_half(NCH - 1, 1)

    nc.compile()
    return nc


_CACHE = {}


def _get_nc():
    if "nc" not in _CACHE:
        _CACHE["nc"] = build_program()
    return _CACHE["nc"]


def _host_prep(Wq, Wk, Wv, Wproj, bproj, dwc_w, dwc_b, scale):
    sc = np.logaddexp(0.0, scale.reshape(C).astype(np.float64)).astype(np.float32)
    w25 = dwc_w.reshape(C, KS * KS)
    w26 = np.concatenate([w25, dwc_b.reshape(C, 1)], axis=1)  # 26th tap = bias
    pairs = PAIRS_EVEN + PAIRS_ODD
    d8 = np.zeros((128, CT, NPAIR, 2, 128), dtype=np.float32)
    for ct in range(CT):
        for j, (ta, tb) in enumerate(pairs):
            for i, t in enumerate((ta, tb)):
                for p in range(128):
                    d8[p, ct, j, i, p] = w26[ct * 128 + p, t]
    shared = {
        "wqTs": np.ascontiguousarray(Wq.T / sc[None, :]).astype(BF16NP),
        "wkTs": np.ascontiguousarray(Wk.T / sc[None, :]).astype(BF16NP),
        "wvT": np.ascontiguousarray(Wv.T).astype(BF16NP),
        "wpT": np.ascontiguousarray(Wproj.T).astype(BF16NP),
        "diag8": np.clip(d8, -240, 240).astype(FP8NP).reshape(128, -1),
        "ident": np.eye(128, dtype=np.float32).astype(BF16NP),
        "bprow": bproj.reshape(1, C).astype(BF16NP),
    }
    return shared


def kernel(query, key, value, Wq, Wk, Wv, Wproj, bproj, dwc_w, dwc_b, scale,
           H=64, W=64, **_unused):
    assert int(H) == 64 and int(W) == 64
    shared = _host_prep(np.asarray(Wq, np.float32), np.asarray(Wk, np.float32),
                        np.asarray(Wv, np.float32), np.asarray(Wproj, np.float32),
                        np.asarray(bproj, np.float32), np.asarray(dwc_w, np.float32),
                        np.asarray(dwc_b, np.float32), np.asarray(scale, np.float32))
    query = np.asarray(query, dtype=np.float32)
    key = np.asarray(key, dtype=np.float32)
    value = np.asarray(value, dtype=np.float32)
    in_maps = []
    for b in range(B):
        m = dict(shared)
        m["qT"] = np.ascontiguousarray(query[b].T).astype(BF16NP)
        m["kT8"] = np.ascontiguousarray(np.clip(key[b].T, -240, 240)).astype(FP8NP)
        m["v8"] = np.ascontiguousarray(
            np.clip(value[b], -240, 240).reshape(NT, 128, C).transpose(1, 0, 2)
            .reshape(128, NT * C)).astype(FP8NP)
        in_maps.append(m)
    nc = _get_nc()
    trace = os.environ.get("KERNEL_PROFILE") == "1"
    kw = {}
    if trace:
        kw["trace"] = True
        d = os.environ.get("KERNEL_PROFILE_DIR")
        if d:
            os.makedirs(d, exist_ok=True)
            kw["tmpdir"] = d
    try:
        res = run_bass_kernel_spmd(nc, in_maps, list(range(B)), **kw)
    except ModuleNotFoundError:
        # NTFF profile hook not available in this container; run untraced
        kw.pop("trace", None)
        kw.pop("tmpdir", None)
        res = run_bass_kernel_spmd(nc, in_maps, list(range(B)), **kw)
    _CACHE["last_res"] = res
    if trace and res.exec_time_ns is not None:
        print(f"HW exec time: {res.exec_time_ns} ns")
    out = np.stack([np.asarray(res.results[i]["out"], dtype=np.float32)
                    for i in range(B)])
    return out


# revision 55
# speedup vs baseline: 1.0211x; 1.0211x over previous
"""CrossFocusedLinearAttentionPrune kernel for 8x TRN2 NeuronCores.

Data-parallel over batch B=8: one batch element per core; the small CxC
weights / C-vectors are replicated (host pre-transposed + pre-cast).

Per-core pipeline (v2):
  - host uploads qT/kT channel-major bf16 and v row-major bf16 (no on-device
    input transposes); 1/softplus(scale) folded into Wq/Wk (relu(x)/s ==
    relu(x/s) for s>0; the 1e-6 eps is dropped, far below bf16 resolution)
  - K path row-major: per 128-row tile, proj -> relu -> cube; kv uses the
    reassociation kv = (k3^T v_raw) Wv^T so v needs no projection pass;
    ksum rides along as a ones-column matmul into the same PSUM group
  - Q path channel-major: proj -> relu -> cube -> q3 [c, n]
  - z: ksum replicated along free dim -> one matmul per chunk gives z_num
    broadcast across all 128 partitions; Act Reciprocal(x+eps) -> zrep
  - x channel-major directly (lhsT = kv), eviction fuses the z multiply and
    writes fp8 into the zero-padded 68x68 conv map
  - depthwise 5x5 conv: 13 fp8 DoubleRow matmuls per c-block (tap pairs via
    a custom strided AP; 26th tap = dwc bias against an all-ones region)
  - h = conv + q3 (gpsimd); out = h @ Wproj^T + bproj, bias via a rank-1
    ones matmul, output DMA'd straight from PSUM per 128-row tile
"""

import os

import numpy as np
import ml_dtypes

import concourse.bacc as bacc
import concourse.bass as bass
import concourse.mybir as mybir
import concourse.tile as tile
from concourse.ap import AP
from concourse.bass_utils import run_bass_kernel_spmd

F32 = mybir.dt.float32
BF16 = mybir.dt.bfloat16
FP8 = mybir.dt.float8e4
AF = mybir.ActivationFunctionType
ALU = mybir.AluOpType
PERF2 = mybir.MatmulPerfMode.DoubleRow

B, N, C = 8, 4096, 256
H = W = 64
KS, PAD = 5, 2
HP = H + 2 * PAD          # 68
EPS = 1e-6
CT = 2                    # channel tiles of 128
NCH = 8                   # 512-wide chunks over N
CHUNK = 512
NT = 32                   # 128-row tiles over N
GUARD = 4                 # zero guard before each ct's map region
ONES_OFF = HP * HP        # ones region for the bias tap (within map part)
XFREE = HP * HP + 544     # per-ct free size incl ones region (excl guard)
XTOT = GUARD + XFREE
NPAIR = 13                # 25 taps + bias tap = 13 DoubleRow pairs
BF16NP = ml_dtypes.bfloat16
FP8NP = ml_dtypes.float8_e4m3

# taps t = 0..24 -> (dy, dx) = (t//5 - 2, t%5 - 2); t = 25 -> bias tap.
# DoubleRow windows must be 2-byte aligned in the fp8 map, so taps are
# paired by dx parity: even-dx taps (and the bias tap) stream full-width
# contiguous [272] windows; odd-dx taps go per map-row with a +1 psum
# shift so their bases become even.
TAPS_EVEN = [t for t in range(25) if (t % 5) % 2 == 0] + [25]   # 16
TAPS_ODD = [t for t in range(25) if (t % 5) % 2 == 1]           # 10
PAIRS_EVEN = [(TAPS_EVEN[2 * i], TAPS_EVEN[2 * i + 1]) for i in range(8)]
PAIRS_ODD = [(TAPS_ODD[2 * i], TAPS_ODD[2 * i + 1]) for i in range(5)]


def _tap_base(t, ch, half):
    # window base so that psum col 68*r + 2 + c maps to out pixel
    # (8*ch + 4*half + r, c); base = (i0 + 2 + dy)*68 + dx
    dy, dx = t // 5 - 2, t % 5 - 2
    return (8 * ch + 4 * half + 2 + dy) * HP + dx


def build_program():
    nc = bacc.Bacc("TRN2", target_bir_lowering=False, debug=False,
                   enable_asserts=False, num_devices=8)

    # -------- DRAM tensors (per-core inputs) --------
    qT_d = nc.dram_tensor("qT", [C, N], BF16, kind="ExternalInput").ap()
    kT_d = nc.dram_tensor("kT8", [C, N], FP8, kind="ExternalInput").ap()
    v_d = nc.dram_tensor("v8", [128, NT * C], FP8, kind="ExternalInput").ap()
    wq_d = nc.dram_tensor("wqTs", [C, C], BF16, kind="ExternalInput").ap()
    wk_d = nc.dram_tensor("wkTs", [C, C], BF16, kind="ExternalInput").ap()
    wv_d = nc.dram_tensor("wvT", [C, C], BF16, kind="ExternalInput").ap()
    wp_d = nc.dram_tensor("wpT", [C, C], BF16, kind="ExternalInput").ap()
    d8_d = nc.dram_tensor("diag8", [128, CT * NPAIR * 2 * 128], FP8,
                          kind="ExternalInput").ap()
    id_d = nc.dram_tensor("ident", [128, 128], BF16, kind="ExternalInput").ap()
    bp_d = nc.dram_tensor("bprow", [1, C], BF16, kind="ExternalInput").ap()
    out_d = nc.dram_tensor("out", [N, C], BF16, kind="ExternalOutput").ap()

    v_r = v_d.rearrange("p (nt c) -> p nt c", c=C)
    out_r = out_d.rearrange("(nt p) c -> p nt c", p=128)

    with tile.TileContext(nc) as tc:
        with (
            tc.tile_pool(name="const", bufs=1) as const,
            tc.tile_pool(name="big", bufs=1) as big,
            tc.tile_pool(name="kpool", bufs=6) as kpool,
            tc.tile_pool(name="qpool", bufs=6) as qpool,
            tc.tile_pool(name="zpool", bufs=4) as zpool,
            tc.tile_pool(name="hpool", bufs=4) as hpool,
            tc.tile_pool(name="smal", bufs=1) as smal,
            tc.tile_pool(name="psA", bufs=2, space="PSUM") as psA,
            tc.tile_pool(name="psB", bufs=2, space="PSUM") as psB,
            tc.tile_pool(name="psC", bufs=2, space="PSUM") as psC,
            tc.tile_pool(name="psKV", bufs=1, space="PSUM") as psKV,
            tc.tile_pool(name="psS", bufs=1, space="PSUM") as psS,
        ):
            # -------- K/V-critical constants, then inputs, then the rest ----
            wk_sb = const.tile([128, CT, C], BF16)
            nc.sync.dma_start(wk_sb[:], wk_d.rearrange("(ct p) d -> p ct d", p=128))
            wq_sb = const.tile([128, CT, C], BF16)
            nc.sync.dma_start(wq_sb[:], wq_d.rearrange("(ct p) d -> p ct d", p=128))

            kT_sb = big.tile([128, CT, N], FP8)
            qT_sb = big.tile([128, CT, N], BF16)
            v_sb = big.tile([128, NT, C], FP8)
            kT_r = kT_d.rearrange("(ct p) n -> p ct n", p=128)
            qT_r = qT_d.rearrange("(ct p) n -> p ct n", p=128)
            kt_cuts = [0, 256, 512] + [512 * i for i in range(2, 9)]
            for i in range(len(kt_cuts) - 1):
                s = slice(kt_cuts[i], kt_cuts[i + 1])
                nc.sync.dma_start(kT_sb[:, :, s], kT_r[:, :, s])
                j = i - 1
                if 0 <= j < 8:
                    nc.sync.dma_start(v_sb[:, 4 * j:4 * j + 4, :],
                                      v_r[:, 4 * j:4 * j + 4, :])
                if 4 <= j < 8:
                    s2 = slice((j - 4) * N // 8, (j - 3) * N // 8)
                    nc.sync.dma_start(qT_sb[:, :, s2], qT_r[:, :, s2])
            for i in range(4, 8):
                s = slice(i * N // 8, (i + 1) * N // 8)
                nc.sync.dma_start(qT_sb[:, :, s], qT_r[:, :, s])

            wv_sb = const.tile([128, CT, C], BF16)
            nc.sync.dma_start(wv_sb[:], wv_d.rearrange("(ct p) d -> p ct d", p=128))
            wp_sb = const.tile([128, CT, C], BF16)
            nc.sync.dma_start(wp_sb[:], wp_d.rearrange("(ct p) d -> p ct d", p=128))
            d8_sb = const.tile([128, CT * NPAIR * 2 * 128], FP8)
            nc.sync.dma_start(d8_sb[:], d8_d)
            id_sb = const.tile([128, 128], BF16)
            nc.sync.dma_start(id_sb[:], id_d)
            bp_sb = const.tile([1, C], BF16)
            nc.sync.dma_start(bp_sb[:], bp_d)

            ones1 = smal.tile([1, 128], BF16)
            nc.vector.memset(ones1[:], 1.0)
            onesk = smal.tile([128, 2, 1], FP8)
            nc.vector.memset(onesk[:], 1.0)
            ones128 = smal.tile([128, 128], BF16)
            nc.vector.memset(ones128[:], 1.0)
            epsrow = smal.tile([1, CHUNK], BF16)
            nc.vector.memset(epsrow[:], EPS)

            # -------- persistent tensors --------
            q3 = big.tile([128, CT, N], BF16)
            xpad = big.tile([128, CT, XTOT], FP8)
            kv_sb = smal.tile([128, CT, C], BF16)
            ksum_bf = smal.tile([128, CT], F32)
            ksr = smal.tile([128, CT, 128], BF16)

            # conv-map halo zeros + ones region (borders only; interior is
            # fully overwritten by the x-phase evictions)
            xmaps = [xpad[:, dt, GUARD:GUARD + HP * HP]
                     .rearrange("p (r c) -> p r c", c=HP) for dt in range(CT)]
            for dt in range(CT):
                xm = xmaps[dt]
                nc.vector.memset(xpad[:, dt, 0:GUARD], 0.0)         # guard
                nc.vector.memset(xm[:, 0:2, :], 0.0)                # top rows
                nc.vector.memset(xm[:, 2 + H:2 + H + 2, :], 0.0)    # bottom
                nc.vector.memset(xm[:, 2:2 + H, 0:2], 0.0)          # left
                nc.vector.memset(xm[:, 2:2 + H, 2 + W:HP], 0.0)     # right
                nc.vector.memset(xpad[:, dt, GUARD + ONES_OFF:XTOT], 1.0)

            kv_ps = psKV.tile([128, CT, C], F32, name="kvps")
            ks_ps = psS.tile([128, CT, 1], F32, name="ksps")

            # ============ K/V phase (row-major) + interleaved Q ============
            def q_chunk(ch):
                # back-half chunks run during the PE-bound conv phase, where
                # DVE is loaded but gpsimd is idle
                eng = nc.vector if ch < 4 else nc.gpsimd
                for dt in range(CT):
                    qps = psA.tile([128, CHUNK], F32, tag="a")
                    for ct in range(CT):
                        nc.tensor.matmul(qps[:], lhsT=wq_sb[:, ct, dt * 128:(dt + 1) * 128],
                                         rhs=qT_sb[:, ct, ch * CHUNK:(ch + 1) * CHUNK],
                                         start=(ct == 0), stop=(ct == 1))
                    mq = qpool.tile([128, CHUNK], BF16, tag="mq")
                    nc.scalar.activation(mq[:], qps[:], AF.Relu)
                    mq2 = qpool.tile([128, CHUNK], BF16, tag="mq2")
                    eng.tensor_tensor(mq2[:], mq[:], mq[:], op=ALU.mult)
                    eng.tensor_tensor(q3[:, dt, ch * CHUNK:(ch + 1) * CHUNK],
                                      mq2[:], mq[:], op=ALU.mult)

            k3_map = {}

            def kv_pair(mm):
                # kv_ps accumulates tmpT[e, c] = sum_m v[m, e] k3[m, c] (v as
                # stationary), so the Wv fix-up needs no transposes at all
                k3 = k3_map.pop(mm)
                for eb in range(CT):
                    nc.tensor.matmul(kv_ps[:, eb, :],
                                     lhsT=v_sb[:, 2 * mm:2 * mm + 2,
                                               eb * 128:(eb + 1) * 128],
                                     rhs=k3[:],
                                     start=(mm == 0), stop=(mm == NT // 2 - 1),
                                     perf_mode=PERF2)
                    nc.tensor.matmul(ks_ps[:, eb, :],
                                     lhsT=k3[:, :, eb * 128:(eb + 1) * 128],
                                     rhs=onesk[:],
                                     start=(mm == 0), stop=(mm == NT // 2 - 1),
                                     perf_mode=PERF2)

            # K processed two 128-row tiles at a time (one [128, 512] psum) to
            # halve the relu/cube op count
            for mm in range(NT // 2):
                kps = psB.tile([128, 2, C], F32, tag="b")
                for g in range(2):
                    m = 2 * mm + g
                    for ct in range(CT):
                        nc.tensor.matmul(kps[:, g, :],
                                         lhsT=kT_sb[:, ct, m * 128:(m + 1) * 128],
                                         rhs=wk_sb[:, ct, :], start=(ct == 0), stop=(ct == 1))
                mk = kpool.tile([128, 2 * C], BF16, tag="mk")
                nc.scalar.activation(mk[:], kps.rearrange("p g c -> p (g c)"), AF.Relu)
                mk2 = kpool.tile([128, 2 * C], BF16, tag="mk2")
                nc.vector.tensor_tensor(mk2[:], mk[:], mk[:], op=ALU.mult)
                k3 = kpool.tile([128, 2, C], FP8, tag="k3", name=f"k3_{mm}")
                k3eng = nc.gpsimd if mm % 2 == 0 else nc.vector
                k3eng.tensor_tensor(k3.rearrange("p g c -> p (g c)"),
                                    mk2[:], mk[:], op=ALU.mult)
                k3_map[mm] = k3
                # kv/ksum contraction lags one mm-step so the relu/cube chain
                # never gates PE; two m-tiles per DoubleRow matmul
                if mm >= 1:
                    kv_pair(mm - 1)
                if mm % 2 == 1 and mm // 2 < 3:
                    q_chunk(mm // 2)
            kv_pair(NT // 2 - 1)

            # ============ ksum replicate + kv fix-up ============
            # q_chunk(3) lands here so PE has work under the Act/DVE links
            nc.scalar.copy(ksum_bf[:], ks_ps.rearrange("p ct one -> p (ct one)"))
            q_chunk(3)
            for dt in range(CT):
                nc.vector.tensor_scalar(ksr[:, dt, :], ones128[:],
                                        ksum_bf[:, dt:dt + 1], None, op0=ALU.mult)

            zrep_map = {}

            def z_chunk(ch):
                zps = psB.tile([128, CHUNK], F32, tag="b")
                nc.tensor.matmul(zps[:], lhsT=ones1[:], rhs=epsrow[:],
                                 start=True, stop=False)
                for ct in range(CT):
                    nc.tensor.matmul(zps[:], lhsT=ksr[:, ct, :],
                                     rhs=q3[:, ct, ch * CHUNK:(ch + 1) * CHUNK],
                                     start=False, stop=(ct == 1))
                zrep = zpool.tile([128, CHUNK], BF16, tag="z", name=f"z{ch}")
                with nc.allow_low_precision(reason="z broadcast, conv-term only"):
                    nc.vector.reciprocal(zrep[:], zps[:])
                zrep_map[ch] = zrep

            # z for the first chunks front-runs the kv fix-up chain so PE
            # stays busy during its Act/DVE links
            z_chunk(0)
            z_chunk(1)

            tmpT = smal.tile([128, CT, C], BF16)   # [e, eb, c]
            for eb in range(CT):
                nc.scalar.copy(tmpT[:, eb, :], kv_ps[:, eb, :])
            for cb in range(CT):
                kvps = psA.tile([128, C], F32, tag="a")
                for eb in range(CT):
                    nc.tensor.matmul(kvps[:], lhsT=tmpT[:, eb, cb * 128:(cb + 1) * 128],
                                     rhs=wv_sb[:, eb, :], start=(eb == 0), stop=(eb == 1))
                nc.scalar.copy(kv_sb[:, cb, :], kvps[:])

            # ============ z + x + conv + proj pipeline ============
            xpad_h = xpad[:, 0, 0:1]   # handle for custom-stride APs
            PSTRIDE = CT * XTOT

            def _pair_rhs(o0, o1, width):
                return AP(xpad_h.tensor, o0,
                          [[PSTRIDE, 128], [o1 - o0, 2], [1, width]])

            hch_map = {}

            def conv_half(ch, half):
                if half == 0:
                    hch_map[ch] = hpool.tile([128, CT, CHUNK], BF16, tag="h",
                                             name=f"h{ch}")
                hch = hch_map[ch]
                for dt in range(CT):
                    if True:
                        base = dt * XTOT + GUARD
                        cps = psC.tile([128, 273], F32, tag="cv")

                        def off(t):
                            if t == 25:
                                return base + ONES_OFF
                            return base + _tap_base(t, ch, half)

                        nmm = 8 + 5 * 4
                        i = 0
                        for j, (ta, tb) in enumerate(PAIRS_EVEN):
                            lhsT = d8_sb[:, (dt * NPAIR + j) * 256:
                                         (dt * NPAIR + j + 1) * 256]
                            nc.tensor.matmul(
                                cps[:, 0:272], lhsT=lhsT.rearrange("p (two m) -> p two m", two=2),
                                rhs=_pair_rhs(off(ta), off(tb), 272),
                                start=(i == 0), stop=(i == nmm - 1),
                                perf_mode=PERF2, skip_group_check=True)
                            i += 1
                        for j, (ta, tb) in enumerate(PAIRS_ODD):
                            lhsT = d8_sb[:, (dt * NPAIR + 8 + j) * 256:
                                         (dt * NPAIR + 8 + j + 1) * 256]
                            lv = lhsT.rearrange("p (two m) -> p two m", two=2)
                            for r in range(4):
                                sh = HP * r + 1
                                nc.tensor.matmul(
                                    cps[:, sh:sh + HP], lhsT=lv,
                                    rhs=_pair_rhs(off(ta) + sh, off(tb) + sh, HP),
                                    start=False, stop=(i == nmm - 1),
                                    perf_mode=PERF2, skip_group_check=True)
                                i += 1
                        cv = cps[:, 0:272].rearrange("p (r c) -> p r c", c=HP)
                        hv = hch[:, dt, half * 256:(half + 1) * 256] \
                            .rearrange("p (r c) -> p r c", c=W)
                        qv = q3[:, dt, ch * CHUNK + half * 256:
                                ch * CHUNK + (half + 1) * 256] \
                            .rearrange("p (r c) -> p r c", c=W)
                        nc.vector.tensor_tensor(hv, cv[:, :, 2:2 + W], qv, op=ALU.add)

            def proj_half(ch, half):
                hch = hch_map[ch] if half == 0 else hch_map.pop(ch)
                ostage = hpool.tile([128, 2, C], BF16, tag="os")
                last = (ch == NCH - 1)
                for g in (2 * half, 2 * half + 1):
                    ops = psB.tile([128, C], F32, tag="b")
                    nc.tensor.matmul(ops[:], lhsT=ones1[:], rhs=bp_sb[:],
                                     start=True, stop=False)
                    for ct in range(CT):
                        nc.tensor.matmul(ops[:], lhsT=hch[:, ct, g * 128:(g + 1) * 128],
                                         rhs=wp_sb[:, ct, :], start=False, stop=(ct == 1))
                    nc.scalar.copy(ostage[:, g - 2 * half, :], ops[:])
                    if last:
                        nc.sync.dma_start(out_r[:, 4 * ch + g, :],
                                          ostage[:, g - 2 * half, :])
                if not last:
                    nc.sync.dma_start(out_r[:, 4 * ch + 2 * half:4 * ch + 2 * half + 2, :],
                                      ostage[:])

            for ch in range(NCH):
                if ch + 4 < NCH:
                    q_chunk(ch + 4)   # q3 for the back half, off the A/B phase
                zrep = zrep_map.pop(ch)
                zv = zrep.rearrange("p (r c) -> p r c", c=W)
                for dt in range(CT):
                    xps = psA.tile([128, CHUNK], F32, tag="a")
                    for ct in range(CT):
                        nc.tensor.matmul(xps[:], lhsT=kv_sb[:, ct, dt * 128:(dt + 1) * 128],
                                         rhs=q3[:, ct, ch * CHUNK:(ch + 1) * CHUNK],
                                         start=(ct == 0), stop=(ct == 1))
                    nc.vector.tensor_tensor(
                        xmaps[dt][:, 2 + 8 * ch:2 + 8 * ch + 8, 2:2 + W],
                        xps.rearrange("p (r c) -> p r c", c=W), zv, op=ALU.mult)
                if ch + 2 < NCH:
                    z_chunk(ch + 2)
                if ch >= 2:
                    conv_half(ch - 2, 1)
                    proj_half(ch - 2, 1)
                if ch >= 1:
                    conv_half(ch - 1, 0)
                    proj_half(ch - 1, 0)
            conv_half(NCH - 2, 1)
            proj_half(NCH - 2, 1)
            conv_half(NCH - 1, 0)
            proj_half(NCH - 1, 0)
            conv_half(NCH - 1, 1)
            proj_half(NCH - 1, 1)

    nc.compile()
    return nc


_CACHE = {}


def _get_nc():
    if "nc" not in _CACHE:
        _CACHE["nc"] = build_program()
    return _CACHE["nc"]


def _host_prep(Wq, Wk, Wv, Wproj, bproj, dwc_w, dwc_b, scale):
    sc = np.logaddexp(0.0, scale.reshape(C).astype(np.float64)).astype(np.float32)
    w25 = dwc_w.reshape(C, KS * KS)
    w26 = np.concatenate([w25, dwc_b.reshape(C, 1)], axis=1)  # 26th tap = bias
    pairs = PAIRS_EVEN + PAIRS_ODD
    d8 = np.zeros((128, CT, NPAIR, 2, 128), dtype=np.float32)
    for ct in range(CT):
        for j, (ta, tb) in enumerate(pairs):
            for i, t in enumerate((ta, tb)):
                for p in range(128):
                    d8[p, ct, j, i, p] = w26[ct * 128 + p, t]
    shared = {
        "wqTs": np.ascontiguousarray(Wq.T / sc[None, :]).astype(BF16NP),
        "wkTs": np.ascontiguousarray(Wk.T / sc[None, :]).astype(BF16NP),
        "wvT": np.ascontiguousarray(Wv.T).astype(BF16NP),
        "wpT": np.ascontiguousarray(Wproj.T).astype(BF16NP),
        "diag8": np.clip(d8, -240, 240).astype(FP8NP).reshape(128, -1),
        "ident": np.eye(128, dtype=np.float32).astype(BF16NP),
        "bprow": bproj.reshape(1, C).astype(BF16NP),
    }
    return shared


def kernel(query, key, value, Wq, Wk, Wv, Wproj, bproj, dwc_w, dwc_b, scale,
           H=64, W=64, **_unused):
    assert int(H) == 64 and int(W) == 64
    shared = _host_prep(np.asarray(Wq, np.float32), np.asarray(Wk, np.float32),
                        np.asarray(Wv, np.float32), np.asarray(Wproj, np.float32),
                        np.asarray(bproj, np.float32), np.asarray(dwc_w, np.float32),
                        np.asarray(dwc_b, np.float32), np.asarray(scale, np.float32))
    query = np.asarray(query, dtype=np.float32)
    key = np.asarray(key, dtype=np.float32)
    value = np.asarray(value, dtype=np.float32)
    in_maps = []
    for b in range(B):
        m = dict(shared)
        m["qT"] = np.ascontiguousarray(query[b].T).astype(BF16NP)
        m["kT8"] = np.ascontiguousarray(np.clip(key[b].T, -240, 240)).astype(FP8NP)
        m["v8"] = np.ascontiguousarray(
            np.clip(value[b], -240, 240).reshape(NT, 128, C).transpose(1, 0, 2)
            .reshape(128, NT * C)).astype(FP8NP)
        in_maps.append(m)
    nc = _get_nc()
    trace = os.environ.get("KERNEL_PROFILE") == "1"
    kw = {}
    if trace:
        kw["trace"] = True
        d = os.environ.get("KERNEL_PROFILE_DIR")
        if d:
            os.makedirs(d, exist_ok=True)
            kw["tmpdir"] = d
    try:
        res = run_bass_kernel_spmd(nc, in_maps, list(range(B)), **kw)
    except ModuleNotFoundError:
        # NTFF profile hook not available in this container; run untraced
        kw.pop("trace", None)
        kw.pop("tmpdir", None)
        res = run_bass_kernel_spmd(nc, in_maps, list(range(B)), **kw)
    _CACHE["last_res"] = res
    if trace and res.exec_time_ns is not None:
        print(f"HW exec time: {res.exec_time_ns} ns")
    out = np.stack([np.asarray(res.results[i]["out"], dtype=np.float32)
                    for i in range(B)])
    return out


# revision 60
# speedup vs baseline: 1.0785x; 1.0562x over previous
"""CrossFocusedLinearAttentionPrune kernel for 8x TRN2 NeuronCores.

Data-parallel over batch B=8: one batch element per core; the small CxC
weights / C-vectors are replicated (host pre-transposed + pre-cast).

Per-core pipeline (v2):
  - host uploads qT/kT channel-major bf16 and v row-major bf16 (no on-device
    input transposes); 1/softplus(scale) folded into Wq/Wk (relu(x)/s ==
    relu(x/s) for s>0; the 1e-6 eps is dropped, far below bf16 resolution)
  - K path row-major: per 128-row tile, proj -> relu -> cube; kv uses the
    reassociation kv = (k3^T v_raw) Wv^T so v needs no projection pass;
    ksum rides along as a ones-column matmul into the same PSUM group
  - Q path channel-major: proj -> relu -> cube -> q3 [c, n]
  - z: ksum replicated along free dim -> one matmul per chunk gives z_num
    broadcast across all 128 partitions; Act Reciprocal(x+eps) -> zrep
  - x channel-major directly (lhsT = kv), eviction fuses the z multiply and
    writes fp8 into the zero-padded 68x68 conv map
  - depthwise 5x5 conv: 13 fp8 DoubleRow matmuls per c-block (tap pairs via
    a custom strided AP; 26th tap = dwc bias against an all-ones region)
  - h = conv + q3 (gpsimd); out = h @ Wproj^T + bproj, bias via a rank-1
    ones matmul, output DMA'd straight from PSUM per 128-row tile
"""

import os

import numpy as np
import ml_dtypes

import concourse.bacc as bacc
import concourse.bass as bass
import concourse.mybir as mybir
import concourse.tile as tile
from concourse.ap import AP
from concourse.bass_utils import run_bass_kernel_spmd

F32 = mybir.dt.float32
BF16 = mybir.dt.bfloat16
FP8 = mybir.dt.float8e4
AF = mybir.ActivationFunctionType
ALU = mybir.AluOpType
PERF2 = mybir.MatmulPerfMode.DoubleRow

B, N, C = 8, 4096, 256
H = W = 64
KS, PAD = 5, 2
HP = H + 2 * PAD          # 68
EPS = 1e-6
CT = 2                    # channel tiles of 128
NCH = 8                   # 512-wide chunks over N
CHUNK = 512
NT = 32                   # 128-row tiles over N
GUARD = 4                 # zero guard before each ct's map region
ONES_OFF = HP * HP        # ones region for the bias tap (within map part)
XFREE = HP * HP + 544     # per-ct free size incl ones region (excl guard)
XTOT = GUARD + XFREE
NPAIR = 13                # 25 taps + bias tap = 13 DoubleRow pairs
BF16NP = ml_dtypes.bfloat16
FP8NP = ml_dtypes.float8_e4m3

# taps t = 0..24 -> (dy, dx) = (t//5 - 2, t%5 - 2); t = 25 -> bias tap.
# DoubleRow windows must be 2-byte aligned in the fp8 map, so taps are
# paired by dx parity: even-dx taps (and the bias tap) stream full-width
# contiguous [272] windows; odd-dx taps go per map-row with a +1 psum
# shift so their bases become even.
TAPS_EVEN = [t for t in range(25) if (t % 5) % 2 == 0] + [25]   # 16
TAPS_ODD = [t for t in range(25) if (t % 5) % 2 == 1]           # 10
PAIRS_EVEN = [(TAPS_EVEN[2 * i], TAPS_EVEN[2 * i + 1]) for i in range(8)]
PAIRS_ODD = [(TAPS_ODD[2 * i], TAPS_ODD[2 * i + 1]) for i in range(5)]


def _tap_base(t, ch, half):
    # window base so that psum col 68*r + 2 + c maps to out pixel
    # (8*ch + 4*half + r, c); base = (i0 + 2 + dy)*68 + dx
    dy, dx = t // 5 - 2, t % 5 - 2
    return (8 * ch + 4 * half + 2 + dy) * HP + dx


def build_program():
    nc = bacc.Bacc("TRN2", target_bir_lowering=False, debug=False,
                   enable_asserts=False, num_devices=8)

    # -------- DRAM tensors (per-core inputs) --------
    qT_d = nc.dram_tensor("qT", [C, N], BF16, kind="ExternalInput").ap()
    kT_d = nc.dram_tensor("kT8", [C, N], FP8, kind="ExternalInput").ap()
    v_d = nc.dram_tensor("v8", [128, NT * C], FP8, kind="ExternalInput").ap()
    wq_d = nc.dram_tensor("wqTs", [C, C], BF16, kind="ExternalInput").ap()
    wk_d = nc.dram_tensor("wkTs", [C, C], BF16, kind="ExternalInput").ap()
    wv_d = nc.dram_tensor("wvT", [C, C], BF16, kind="ExternalInput").ap()
    wp_d = nc.dram_tensor("wpT", [C, C], BF16, kind="ExternalInput").ap()
    d8_d = nc.dram_tensor("diag8", [128, CT * NPAIR * 2 * 128], FP8,
                          kind="ExternalInput").ap()
    bp_d = nc.dram_tensor("bprep", [128, C], BF16, kind="ExternalInput").ap()
    out_d = nc.dram_tensor("out", [N, C], BF16, kind="ExternalOutput").ap()

    v_r = v_d.rearrange("p (nt c) -> p nt c", c=C)
    out_r = out_d.rearrange("(nt p) c -> p nt c", p=128)

    with tile.TileContext(nc) as tc:
        with (
            tc.tile_pool(name="const", bufs=1) as const,
            tc.tile_pool(name="big", bufs=1) as big,
            tc.tile_pool(name="kpool", bufs=6) as kpool,
            tc.tile_pool(name="qpool", bufs=6) as qpool,
            tc.tile_pool(name="zpool", bufs=4) as zpool,
            tc.tile_pool(name="hpool", bufs=4) as hpool,
            tc.tile_pool(name="smal", bufs=1) as smal,
            tc.tile_pool(name="psA", bufs=2, space="PSUM") as psA,
            tc.tile_pool(name="psB", bufs=2, space="PSUM") as psB,
            tc.tile_pool(name="psC", bufs=2, space="PSUM") as psC,
            tc.tile_pool(name="psKV", bufs=1, space="PSUM") as psKV,
            tc.tile_pool(name="psS", bufs=1, space="PSUM") as psS,
        ):
            # -------- K/V-critical constants, then inputs, then the rest ----
            wk_sb = const.tile([128, CT, C], BF16)
            nc.sync.dma_start(wk_sb[:], wk_d.rearrange("(ct p) d -> p ct d", p=128))
            wq_sb = const.tile([128, CT, C], BF16)
            nc.sync.dma_start(wq_sb[:], wq_d.rearrange("(ct p) d -> p ct d", p=128))

            kT_sb = big.tile([128, CT, N], FP8)
            qT_sb = big.tile([128, CT, N], BF16)
            v_sb = big.tile([128, NT, C], FP8)
            kT_r = kT_d.rearrange("(ct p) n -> p ct n", p=128)
            qT_r = qT_d.rearrange("(ct p) n -> p ct n", p=128)
            kt_cuts = [0, 256, 512] + [512 * i for i in range(2, 9)]
            for i in range(len(kt_cuts) - 1):
                s = slice(kt_cuts[i], kt_cuts[i + 1])
                nc.sync.dma_start(kT_sb[:, :, s], kT_r[:, :, s])
                j = i - 1
                if 0 <= j < 8:
                    nc.sync.dma_start(v_sb[:, 4 * j:4 * j + 4, :],
                                      v_r[:, 4 * j:4 * j + 4, :])
                if 4 <= j < 8:
                    s2 = slice((j - 4) * N // 8, (j - 3) * N // 8)
                    nc.sync.dma_start(qT_sb[:, :, s2], qT_r[:, :, s2])
            for i in range(4, 8):
                s = slice(i * N // 8, (i + 1) * N // 8)
                nc.sync.dma_start(qT_sb[:, :, s], qT_r[:, :, s])

            wv_sb = const.tile([128, CT, C], BF16)
            nc.sync.dma_start(wv_sb[:], wv_d.rearrange("(ct p) d -> p ct d", p=128))
            wp_sb = const.tile([128, CT, C], BF16)
            nc.sync.dma_start(wp_sb[:], wp_d.rearrange("(ct p) d -> p ct d", p=128))
            d8_sb = const.tile([128, CT * NPAIR * 2 * 128], FP8)
            nc.sync.dma_start(d8_sb[:], d8_d)
            bp_sb = const.tile([128, C], BF16)
            nc.sync.dma_start(bp_sb[:], bp_d)

            onesk = smal.tile([128, 2, 1], FP8)
            nc.vector.memset(onesk[:], 1.0)
            ones128 = smal.tile([128, 128], BF16)
            nc.vector.memset(ones128[:], 1.0)

            # -------- persistent tensors --------
            q3 = big.tile([128, CT, N], BF16)
            xpad = big.tile([128, CT, XTOT], FP8)
            kv_sb = smal.tile([128, CT, C], BF16)
            ksum_bf = smal.tile([128, CT], F32)
            ksr = smal.tile([128, CT, 128], BF16)

            # conv-map halo zeros + ones region (borders only; interior is
            # fully overwritten by the x-phase evictions)
            xmaps = [xpad[:, dt, GUARD:GUARD + HP * HP]
                     .rearrange("p (r c) -> p r c", c=HP) for dt in range(CT)]
            for dt in range(CT):
                xm = xmaps[dt]
                nc.vector.memset(xpad[:, dt, 0:GUARD], 0.0)         # guard
                nc.vector.memset(xm[:, 0:2, :], 0.0)                # top rows
                nc.vector.memset(xm[:, 2 + H:2 + H + 2, :], 0.0)    # bottom
                nc.vector.memset(xm[:, 2:2 + H, 0:2], 0.0)          # left
                nc.vector.memset(xm[:, 2:2 + H, 2 + W:HP], 0.0)     # right
                nc.vector.memset(xpad[:, dt, GUARD + ONES_OFF:XTOT], 1.0)

            kv_ps = psKV.tile([128, CT, C], F32, name="kvps")
            ks_ps = psS.tile([128, CT, 1], F32, name="ksps")

            # ============ K/V phase (row-major) + interleaved Q ============
            def q_chunk(ch):
                # back-half chunks run during the PE-bound conv phase, where
                # DVE is loaded but gpsimd is idle
                eng = nc.vector if ch < 4 else nc.gpsimd
                for dt in range(CT):
                    qps = psA.tile([128, CHUNK], F32, tag="a")
                    for ct in range(CT):
                        nc.tensor.matmul(qps[:], lhsT=wq_sb[:, ct, dt * 128:(dt + 1) * 128],
                                         rhs=qT_sb[:, ct, ch * CHUNK:(ch + 1) * CHUNK],
                                         start=(ct == 0), stop=(ct == 1))
                    mq = qpool.tile([128, CHUNK], BF16, tag="mq")
                    nc.scalar.activation(mq[:], qps[:], AF.Relu)
                    mq2 = qpool.tile([128, CHUNK], BF16, tag="mq2")
                    eng.tensor_tensor(mq2[:], mq[:], mq[:], op=ALU.mult)
                    eng.tensor_tensor(q3[:, dt, ch * CHUNK:(ch + 1) * CHUNK],
                                      mq2[:], mq[:], op=ALU.mult)

            k3_map = {}

            def kv_pair(mm):
                # kv_ps accumulates tmpT[e, c] = sum_m v[m, e] k3[m, c] (v as
                # stationary), so the Wv fix-up needs no transposes at all
                k3 = k3_map.pop(mm)
                for eb in range(CT):
                    nc.tensor.matmul(kv_ps[:, eb, :],
                                     lhsT=v_sb[:, 2 * mm:2 * mm + 2,
                                               eb * 128:(eb + 1) * 128],
                                     rhs=k3[:],
                                     start=(mm == 0), stop=(mm == NT // 2 - 1),
                                     perf_mode=PERF2)
                    nc.tensor.matmul(ks_ps[:, eb, :],
                                     lhsT=k3[:, :, eb * 128:(eb + 1) * 128],
                                     rhs=onesk[:],
                                     start=(mm == 0), stop=(mm == NT // 2 - 1),
                                     perf_mode=PERF2)

            # K processed two 128-row tiles at a time (one [128, 512] psum) to
            # halve the relu/cube op count
            for mm in range(NT // 2):
                kps = psB.tile([128, 2, C], F32, tag="b")
                for g in range(2):
                    m = 2 * mm + g
                    for ct in range(CT):
                        nc.tensor.matmul(kps[:, g, :],
                                         lhsT=kT_sb[:, ct, m * 128:(m + 1) * 128],
                                         rhs=wk_sb[:, ct, :], start=(ct == 0), stop=(ct == 1))
                mk = kpool.tile([128, 2 * C], BF16, tag="mk")
                nc.scalar.activation(mk[:], kps.rearrange("p g c -> p (g c)"), AF.Relu)
                mk2 = kpool.tile([128, 2 * C], BF16, tag="mk2")
                nc.vector.tensor_tensor(mk2[:], mk[:], mk[:], op=ALU.mult)
                k3 = kpool.tile([128, 2, C], FP8, tag="k3", name=f"k3_{mm}")
                k3eng = nc.gpsimd if mm % 2 == 0 else nc.vector
                k3eng.tensor_tensor(k3.rearrange("p g c -> p (g c)"),
                                    mk2[:], mk[:], op=ALU.mult)
                k3_map[mm] = k3
                # kv/ksum contraction lags one mm-step so the relu/cube chain
                # never gates PE; two m-tiles per DoubleRow matmul
                if mm >= 1:
                    kv_pair(mm - 1)
                if mm % 2 == 1 and mm // 2 < 3:
                    q_chunk(mm // 2)
            kv_pair(NT // 2 - 1)

            # ============ ksum replicate + kv fix-up ============
            # q_chunk(3) lands here so PE has work under the Act/DVE links
            nc.scalar.copy(ksum_bf[:], ks_ps.rearrange("p ct one -> p (ct one)"))
            q_chunk(3)
            for dt in range(CT):
                nc.vector.tensor_scalar(ksr[:, dt, :], ones128[:],
                                        ksum_bf[:, dt:dt + 1], None, op0=ALU.mult)

            zrep_map = {}

            def z_chunk(ch):
                # z_num = q3 . ksum >= ~1e-6 always (nonneg cubes, large
                # ksum), so the reference's +1e-6 is numerically invisible
                # and is dropped
                zps = psB.tile([128, CHUNK], F32, tag="b")
                for ct in range(CT):
                    nc.tensor.matmul(zps[:], lhsT=ksr[:, ct, :],
                                     rhs=q3[:, ct, ch * CHUNK:(ch + 1) * CHUNK],
                                     start=(ct == 0), stop=(ct == 1))
                zrep = zpool.tile([128, CHUNK], BF16, tag="z", name=f"z{ch}")
                with nc.allow_low_precision(reason="z broadcast, conv-term only"):
                    nc.vector.reciprocal(zrep[:], zps[:])
                zrep_map[ch] = zrep

            # z for the first chunks front-runs the kv fix-up chain so PE
            # stays busy during its Act/DVE links
            z_chunk(0)
            z_chunk(1)

            tmpT = smal.tile([128, CT, C], BF16)   # [e, eb, c]
            for eb in range(CT):
                nc.scalar.copy(tmpT[:, eb, :], kv_ps[:, eb, :])
            for cb in range(CT):
                kvps = psA.tile([128, C], F32, tag="a")
                for eb in range(CT):
                    nc.tensor.matmul(kvps[:], lhsT=tmpT[:, eb, cb * 128:(cb + 1) * 128],
                                     rhs=wv_sb[:, eb, :], start=(eb == 0), stop=(eb == 1))
                nc.scalar.copy(kv_sb[:, cb, :], kvps[:])

            # ============ z + x + conv + proj pipeline ============
            xpad_h = xpad[:, 0, 0:1]   # handle for custom-stride APs
            PSTRIDE = CT * XTOT

            def _pair_rhs(o0, o1, width):
                return AP(xpad_h.tensor, o0,
                          [[PSTRIDE, 128], [o1 - o0, 2], [1, width]])

            hch_map = {}

            def conv_half(ch, half):
                if half == 0:
                    hch_map[ch] = hpool.tile([128, CT, CHUNK], BF16, tag="h",
                                             name=f"h{ch}")
                hch = hch_map[ch]
                for dt in range(CT):
                    if True:
                        base = dt * XTOT + GUARD
                        cps = psC.tile([128, 273], F32, tag="cv")

                        def off(t):
                            if t == 25:
                                return base + ONES_OFF
                            return base + _tap_base(t, ch, half)

                        nmm = 8 + 5 * 4
                        i = 0
                        for j, (ta, tb) in enumerate(PAIRS_EVEN):
                            lhsT = d8_sb[:, (dt * NPAIR + j) * 256:
                                         (dt * NPAIR + j + 1) * 256]
                            nc.tensor.matmul(
                                cps[:, 0:272], lhsT=lhsT.rearrange("p (two m) -> p two m", two=2),
                                rhs=_pair_rhs(off(ta), off(tb), 272),
                                start=(i == 0), stop=(i == nmm - 1),
                                perf_mode=PERF2, skip_group_check=True)
                            i += 1
                        for j, (ta, tb) in enumerate(PAIRS_ODD):
                            lhsT = d8_sb[:, (dt * NPAIR + 8 + j) * 256:
                                         (dt * NPAIR + 8 + j + 1) * 256]
                            lv = lhsT.rearrange("p (two m) -> p two m", two=2)
                            for r in range(4):
                                sh = HP * r + 1
                                nc.tensor.matmul(
                                    cps[:, sh:sh + HP], lhsT=lv,
                                    rhs=_pair_rhs(off(ta) + sh, off(tb) + sh, HP),
                                    start=False, stop=(i == nmm - 1),
                                    perf_mode=PERF2, skip_group_check=True)
                                i += 1
                        cv = cps[:, 0:272].rearrange("p (r c) -> p r c", c=HP)
                        hv = hch[:, dt, half * 256:(half + 1) * 256] \
                            .rearrange("p (r c) -> p r c", c=W)
                        qv = q3[:, dt, ch * CHUNK + half * 256:
                                ch * CHUNK + (half + 1) * 256] \
                            .rearrange("p (r c) -> p r c", c=W)
                        nc.vector.tensor_tensor(hv, cv[:, :, 2:2 + W], qv, op=ALU.add)

            def proj_half(ch, half):
                hch = hch_map[ch] if half == 0 else hch_map.pop(ch)
                ostage = hpool.tile([128, 2, C], BF16, tag="os")
                last = (ch == NCH - 1)
                for g in (2 * half, 2 * half + 1):
                    ops = psB.tile([128, C], F32, tag="b")
                    for ct in range(CT):
                        nc.tensor.matmul(ops[:], lhsT=hch[:, ct, g * 128:(g + 1) * 128],
                                         rhs=wp_sb[:, ct, :], start=(ct == 0), stop=(ct == 1))
                    nc.vector.tensor_tensor(ostage[:, g - 2 * half, :], ops[:],
                                            bp_sb[:], op=ALU.add)
                    if last:
                        nc.sync.dma_start(out_r[:, 4 * ch + g, :],
                                          ostage[:, g - 2 * half, :])
                if not last:
                    nc.sync.dma_start(out_r[:, 4 * ch + 2 * half:4 * ch + 2 * half + 2, :],
                                      ostage[:])

            for ch in range(NCH):
                if ch + 4 < NCH:
                    q_chunk(ch + 4)   # q3 for the back half, off the A/B phase
                zrep = zrep_map.pop(ch)
                zv = zrep.rearrange("p (r c) -> p r c", c=W)
                for dt in range(CT):
                    xps = psA.tile([128, CHUNK], F32, tag="a")
                    for ct in range(CT):
                        nc.tensor.matmul(xps[:], lhsT=kv_sb[:, ct, dt * 128:(dt + 1) * 128],
                                         rhs=q3[:, ct, ch * CHUNK:(ch + 1) * CHUNK],
                                         start=(ct == 0), stop=(ct == 1))
                    nc.vector.tensor_tensor(
                        xmaps[dt][:, 2 + 8 * ch:2 + 8 * ch + 8, 2:2 + W],
                        xps.rearrange("p (r c) -> p r c", c=W), zv, op=ALU.mult)
                if ch + 2 < NCH:
                    z_chunk(ch + 2)
                if ch >= 2:
                    conv_half(ch - 2, 1)
                    proj_half(ch - 2, 1)
                if ch >= 1:
                    conv_half(ch - 1, 0)
                    proj_half(ch - 1, 0)
            conv_half(NCH - 2, 1)
            proj_half(NCH - 2, 1)
            conv_half(NCH - 1, 0)
            proj_half(NCH - 1, 0)
            conv_half(NCH - 1, 1)
            proj_half(NCH - 1, 1)

    nc.compile()
    return nc


_CACHE = {}


def _get_nc():
    if "nc" not in _CACHE:
        _CACHE["nc"] = build_program()
    return _CACHE["nc"]


def _host_prep(Wq, Wk, Wv, Wproj, bproj, dwc_w, dwc_b, scale):
    sc = np.logaddexp(0.0, scale.reshape(C).astype(np.float64)).astype(np.float32)
    w25 = dwc_w.reshape(C, KS * KS)
    w26 = np.concatenate([w25, dwc_b.reshape(C, 1)], axis=1)  # 26th tap = bias
    pairs = PAIRS_EVEN + PAIRS_ODD
    d8 = np.zeros((128, CT, NPAIR, 2, 128), dtype=np.float32)
    for ct in range(CT):
        for j, (ta, tb) in enumerate(pairs):
            for i, t in enumerate((ta, tb)):
                for p in range(128):
                    d8[p, ct, j, i, p] = w26[ct * 128 + p, t]
    shared = {
        "wqTs": np.ascontiguousarray(Wq.T / sc[None, :]).astype(BF16NP),
        "wkTs": np.ascontiguousarray(Wk.T / sc[None, :]).astype(BF16NP),
        "wvT": np.ascontiguousarray(Wv.T).astype(BF16NP),
        "wpT": np.ascontiguousarray(Wproj.T).astype(BF16NP),
        "diag8": np.clip(d8, -240, 240).astype(FP8NP).reshape(128, -1),
        "bprep": np.ascontiguousarray(
            np.broadcast_to(bproj.reshape(1, C), (128, C))).astype(BF16NP),
    }
    return shared


def kernel(query, key, value, Wq, Wk, Wv, Wproj, bproj, dwc_w, dwc_b, scale,
           H=64, W=64, **_unused):
    assert int(H) == 64 and int(W) == 64
    shared = _host_prep(np.asarray(Wq, np.float32), np.asarray(Wk, np.float32),
                        np.asarray(Wv, np.float32), np.asarray(Wproj, np.float32),
                        np.asarray(bproj, np.float32), np.asarray(dwc_w, np.float32),
                        np.asarray(dwc_b, np.float32), np.asarray(scale, np.float32))
    query = np.asarray(query, dtype=np.float32)
    key = np.asarray(key, dtype=np.float32)
    value = np.asarray(value, dtype=np.float32)
    in_maps = []
    for b in range(B):
        m = dict(shared)
        m["qT"] = np.ascontiguousarray(query[b].T).astype(BF16NP)
        m["kT8"] = np.ascontiguousarray(np.clip(key[b].T, -240, 240)).astype(FP8NP)
        m["v8"] = np.ascontiguousarray(
            np.clip(value[b], -240, 240).reshape(NT, 128, C).transpose(1, 0, 2)
            .reshape(128, NT * C)).astype(FP8NP)
        in_maps.append(m)
    nc = _get_nc()
    trace = os.environ.get("KERNEL_PROFILE") == "1"
    kw = {}
    if trace:
        kw["trace"] = True
        d = os.environ.get("KERNEL_PROFILE_DIR")
        if d:
            os.makedirs(d, exist_ok=True)
            kw["tmpdir"] = d
    try:
        res = run_bass_kernel_spmd(nc, in_maps, list(range(B)), **kw)
    except ModuleNotFoundError:
        # NTFF profile hook not available in this container; run untraced
        kw.pop("trace", None)
        kw.pop("tmpdir", None)
        res = run_bass_kernel_spmd(nc, in_maps, list(range(B)), **kw)
    _CACHE["last_res"] = res
    if trace and res.exec_time_ns is not None:
        print(f"HW exec time: {res.exec_time_ns} ns")
    out = np.stack([np.asarray(res.results[i]["out"], dtype=np.float32)
                    for i in range(B)])
    return out
